# revision 1
# baseline (speedup 1.0000x reference)
"""Trainium2 Bass kernel for the AdaptiveCSABlock (Swin-style windowed attention
block): B=8, C=192, 224x224 image, 7x7 windows, 6 heads, MLP hidden 768.

Strategy: data-parallel over batch (1 image per NeuronCore, 8 cores).
Host pre-permutes x into window order ([C, 1024 windows, 49 tokens]); the device
kernel is a fully fused channel-major pipeline over 392-token tiles (8 windows):
  LN1 (PE ones-matmul column stats, rsqrt via ACT ln+exp) -> qkv (bf16 matmuls)
  -> per-window attention computed as S^T = K^T(Q) so the softmax reduction
  runs on the PE (ones-matmul denominators) -> proj + residual -> LN2 -> MLP
  with exact-ACT gelu -> residual.  No DRAM round-trips for activations.
Tiles are processed in groups with attention/MLP phase separation so the
ScalarE activation-table set (natural_log_exp vs gelu) switches only twice per
group.
"""
import numpy as np
import ml_dtypes

DIM, WS, NH, MLP_H = 192, 7, 6, 768
B, H, W = 8, 224, 224
N = WS * WS            # 49
HD = DIM // NH         # 32
NWIN = (H // WS) * (W // WS)   # 1024
WPT = 8                # windows per tile
TPT = WPT * N          # 392 tokens per tile
NTILES = NWIN // WPT   # 128
GROUP = 16             # tiles per ACT-table phase group
EPS = 1e-5
NPAIR = WPT // 2

bf16_np = ml_dtypes.bfloat16


def _relative_position_index():
    coords = np.stack(np.meshgrid(np.arange(WS), np.arange(WS), indexing='ij'))
    cf = coords.reshape(2, -1)
    rel = (cf[:, :, None] - cf[:, None, :]).transpose(1, 2, 0).copy()
    rel[:, :, 0] += WS - 1
    rel[:, :, 1] += WS - 1
    rel[:, :, 0] *= 2 * WS - 1
    return rel.sum(-1).astype(np.int32)


def _pad_bias_cols(b, rows=128):
    """[n] -> [rows, ceil(n/rows)] column-chunk layout."""
    n = b.shape[0]
    ncol = (n + rows - 1) // rows
    out = np.zeros((rows, ncol), np.float32)
    for j in range(ncol):
        seg = b[rows * j: rows * (j + 1)]
        out[0:seg.shape[0], j] = seg
    return out


def prep_weights(inp):
    """Host-side weight preprocessing. Returns (dict of np arrays, flags)."""
    f32 = np.float32
    ln1_w = np.asarray(inp['ln1_w'], f32)
    ln1_b = np.asarray(inp['ln1_b'], f32)
    qkv_w = np.asarray(inp['qkv_w'], f32) * ln1_w[:, None]
    qkv_bias = np.asarray(inp['qkv_b'], f32) + np.asarray(inp['qkv_w'], f32).T @ ln1_b
    scale = HD ** -0.5
    Wq = qkv_w[:, 0:DIM] * scale
    Wk = qkv_w[:, DIM:2 * DIM]
    Wv = qkv_w[:, 2 * DIM:3 * DIM]
    bq = qkv_bias[0:DIM] * scale
    bk = qkv_bias[DIM:2 * DIM]
    bv = qkv_bias[2 * DIM:3 * DIM]
    assert np.all(bv == 0.0), "nonzero v bias not supported by this kernel"
    # qkv output-channel grouping: wa=[Q0..Q3], wb=[K0..K3], wcd=[Q4,Q5,K4,K5]
    wa = Wq[:, 0:128]
    wb = Wk[:, 0:128]
    wcd = np.concatenate([Wq[:, 128:192], Wk[:, 128:192]], axis=1)
    # rpb bias tile: bsb[m, 49h+n] = table[rel_idx[n, m], h]
    rel = _relative_position_index()
    table = np.asarray(inp['rpb_table'], f32)
    bias_nmh = table[rel.reshape(-1)].reshape(N, N, NH)
    bsb = np.zeros((N, NH * N), f32)
    for h in range(NH):
        bsb[:, N * h:N * h + N] = bias_nmh[:, :, h].T
    ln2_w = np.asarray(inp['ln2_w'], f32)
    ln2_b = np.asarray(inp['ln2_b'], f32)
    w1 = np.asarray(inp['fc1_w'], f32) * ln2_w[:, None]
    b1 = np.asarray(inp['fc1_b'], f32) + np.asarray(inp['fc1_w'], f32).T @ ln2_b
    w2 = np.asarray(inp['fc2_w'], f32)
    b2 = np.asarray(inp['fc2_b'], f32)
    wp = np.asarray(inp['proj_w'], f32)
    bp = np.asarray(inp['proj_b'], f32)

    bf = bf16_np
    wts = {
        'wa': wa.astype(bf), 'wb': wb.astype(bf), 'wcd': wcd.astype(bf),
        'wv': Wv.astype(bf), 'wp': wp.astype(bf),
        'w1': w1.astype(bf), 'w2': w2.astype(bf),
        'bsb': bsb.astype(bf),
        'i49': np.concatenate([np.eye(N, dtype=f32), np.zeros((N, 15), f32)], 1).astype(bf),
        'ba': _pad_bias_cols(bq[0:128]), 'bb': _pad_bias_cols(bk[0:128]),
        'bcd': _pad_bias_cols(np.concatenate([bq[128:192], bk[128:192]])),
        'b1': _pad_bias_cols(b1), 'b2': _pad_bias_cols(b2),
        'bp': _pad_bias_cols(bp),
    }
    flags = {
        'qkv_bias': bool(np.any(qkv_bias)),
        'fc1_bias': bool(np.any(b1)),
        'fc2_bias': bool(np.any(b2)),
        'proj_bias': bool(np.any(bp)),
    }
    return wts, flags


def build_program(ntiles=NTILES, group=GROUP, flags=None, sim_gelu=False, stage=99):
    from contextlib import ExitStack
    import concourse.bacc as bacc
    import concourse.tile as tile
    from concourse import mybir

    flags = flags or {k: False for k in ('qkv_bias', 'fc1_bias', 'fc2_bias', 'proj_bias')}
    f32 = mybir.dt.float32
    f32r = mybir.dt.float32r
    bf = mybir.dt.bfloat16
    A = mybir.ActivationFunctionType
    GELU_F = A.Identity if sim_gelu else A.Gelu

    ntok = ntiles * TPT
    nc = bacc.Bacc(target_bir_lowering=False, debug=False)
    x_ext = nc.declare_dram_parameter("x", [DIM, ntok], f32, isOutput=False)
    out_ext = nc.declare_dram_parameter("out", [DIM, ntok], f32, isOutput=True)
    wshapes = {
        'wa': ([DIM, 128], bf), 'wb': ([DIM, 128], bf), 'wcd': ([DIM, 128], bf),
        'wv': ([DIM, DIM], bf), 'wp': ([DIM, DIM], bf),
        'w1': ([DIM, MLP_H], bf), 'w2': ([MLP_H, DIM], bf),
        'bsb': ([N, NH * N], bf), 'i49': ([N, 64], bf),
        'ba': ([128, 1], f32), 'bb': ([128, 1], f32), 'bcd': ([128, 1], f32),
        'b1': ([128, 6], f32), 'b2': ([128, 2], f32), 'bp': ([128, 2], f32),
    }
    wext = {k: nc.declare_dram_parameter(k, s, d, isOutput=False)
            for k, (s, d) in wshapes.items()}

    with tile.TileContext(nc) as tc, ExitStack() as ctx:
        consts = ctx.enter_context(tc.tile_pool(name="consts", bufs=1))

        def load_pair(name, cols, dt=bf):
            t1 = consts.tile([128, cols], dt, tag=name + "1")
            t2 = consts.tile([64, cols], dt, tag=name + "2")
            nc.sync.dma_start(out=t1, in_=wext[name][0:128, :])
            nc.sync.dma_start(out=t2, in_=wext[name][128:DIM, :])
            return t1, t2

        wa1, wa2 = load_pair('wa', 128)
        wb1, wb2 = load_pair('wb', 128)
        wcd1, wcd2 = load_pair('wcd', 128)
        wv1, wv2 = load_pair('wv', DIM)
        wp1, wp2 = load_pair('wp', DIM)
        w11, w12 = load_pair('w1', MLP_H)
        w2sb = consts.tile([128, 6 * DIM], bf, tag="w2sb")
        for j in range(6):
            nc.sync.dma_start(out=w2sb[:, DIM * j:DIM * (j + 1)],
                              in_=wext['w2'][128 * j:128 * (j + 1), :])
        bsb_t = consts.tile([N, NH * N], bf, tag="bsb")
        nc.sync.dma_start(out=bsb_t, in_=wext['bsb'][:])
        i49_t = consts.tile([N, 64], bf, tag="i49")
        nc.sync.dma_start(out=i49_t, in_=wext['i49'][:])
        ones_den = consts.tile([128, 32], bf, tag="ones_den")
        nc.gpsimd.memset(ones_den, 1.0)
        onecf = consts.tile([128, 1], f32, tag="onecf")
        nc.gpsimd.memset(onecf, 1.0)
        onec1 = consts.tile([128, 1], f32r, tag="onec1")
        onec2 = consts.tile([64, 1], f32r, tag="onec2")
        nc.gpsimd.tensor_copy(onec1, onecf)
        nc.gpsimd.tensor_copy(onec2, onecf[0:64, :])
        eps_t = consts.tile([128, 1], f32, tag="eps")
        nc.gpsimd.memset(eps_t, EPS)
        biases = {}
        use_bias = {'ba': flags['qkv_bias'], 'bb': flags['qkv_bias'],
                    'bcd': flags['qkv_bias'], 'b1': flags['fc1_bias'],
                    'b2': flags['fc2_bias'], 'bp': flags['proj_bias']}
        for bn, used in use_bias.items():
            if used:
                bt = consts.tile(wshapes[bn][0], f32, tag="bias_" + bn)
                nc.sync.dma_start(out=bt, in_=wext[bn][:])
                biases[bn] = bt

        # --- pools ---
        xp = ctx.enter_context(tc.tile_pool(name="xp", bufs=3))
        work = ctx.enter_context(tc.tile_pool(name="work", bufs=3))
        qkp = ctx.enter_context(tc.tile_pool(name="qkp", bufs=2))
        vp = ctx.enter_context(tc.tile_pool(name="vp", bufs=2 * NPAIR + 1))
        epool = ctx.enter_context(tc.tile_pool(name="epool", bufs=2 * NPAIR + 1))
        aop = ctx.enter_context(tc.tile_pool(name="aop", bufs=4))
        x2p = ctx.enter_context(tc.tile_pool(name="x2p", bufs=group + 2))
        gpool = ctx.enter_context(tc.tile_pool(name="gpool", bufs=2))
        opool = ctx.enter_context(tc.tile_pool(name="opool", bufs=3))
        ps_sb = ctx.enter_context(tc.tile_pool(name="ps_sb", bufs=2, space="PSUM"))
        ps_mm = ctx.enter_context(tc.tile_pool(name="ps_mm", bufs=2, space="PSUM"))
        ps_sp = ctx.enter_context(tc.tile_pool(name="ps_sp", bufs=2, space="PSUM"))
        ps_ao = ctx.enter_context(tc.tile_pool(name="ps_ao", bufs=2, space="PSUM"))

        def ln_stats(src1, src2):
            """Column mean+rstd of the stacked [192, TPT] activation (f32 sbuf
            chunks src1 [128,.], src2 [64,.]). Returns (mu_row, r_row) [1, TPT]."""
            xsq1 = work.tile([128, TPT], f32r, tag="xsq1", bufs=2)
            xsq2 = work.tile([64, TPT], f32r, tag="xsq2", bufs=2)
            nc.vector.tensor_mul(xsq1, src1, src1)
            nc.vector.tensor_mul(xsq2, src2, src2)
            xr1 = work.tile([128, TPT], f32r, tag="xr1", bufs=2)
            xr2 = work.tile([64, TPT], f32r, tag="xr2", bufs=2)
            nc.gpsimd.tensor_copy(xr1, src1)
            nc.gpsimd.tensor_copy(xr2, src2)
            st = ps_sb.tile([128, 512], f32, tag="sbc")
            st2 = ps_sp.tile([128, 512], f32, tag="sp")
            nc.tensor.matmul(st[0:1, 0:TPT], onec1, xr1, start=True, stop=False)
            nc.tensor.matmul(st[0:1, 0:TPT], onec2, xr2, start=False, stop=True)
            nc.tensor.matmul(st2[0:1, 0:TPT], onec1, xsq1, start=True, stop=False)
            nc.tensor.matmul(st2[0:1, 0:TPT], onec2, xsq2, start=False, stop=True)
            rows = work.tile([1, 6 * TPT], f32, tag="rows", bufs=2)

            def _row(j):
                return rows[0:1, j * TPT:(j + 1) * TPT]

            mu_row = _row(0)   # S0
            ve_row = _row(1)   # S1: t1 then ve (in-place)
            sc1 = _row(2)      # S2: mu2 / ib / ta / tb
            sc2 = _row(3)      # S3: z / s
            r0_row = _row(4)   # S4
            r_row = _row(5)    # S5
            nc.vector.tensor_scalar_mul(mu_row, st[0:1, 0:TPT], 1.0 / DIM)
            nc.vector.tensor_scalar(ve_row, st2[0:1, 0:TPT], 1.0 / DIM, EPS,
                                    op0=mybir.AluOpType.mult,
                                    op1=mybir.AluOpType.add)
            nc.vector.tensor_mul(sc1, mu_row, mu_row)
            nc.vector.tensor_sub(ve_row, ve_row, sc1)
            # r ~= exp(-0.5*ln(ve)) with ln via exponent-bit trick, then 1 Newton
            nc.vector.tensor_copy(sc1, ve_row.bitcast(mybir.dt.int32))
            LN2C = 0.69314718056
            nc.vector.tensor_scalar(sc2, sc1, -0.5 * LN2C / (1 << 23),
                                    0.5 * (127.0 - 0.0430) * LN2C,
                                    op0=mybir.AluOpType.mult,
                                    op1=mybir.AluOpType.add)
            nc.scalar.activation(r0_row, sc2, A.Exp)
            nc.vector.tensor_mul(sc1, r0_row, r0_row)
            nc.vector.tensor_mul(sc1, sc1, ve_row)
            nc.vector.tensor_scalar(sc2, sc1, -0.5, 1.5,
                                    op0=mybir.AluOpType.mult,
                                    op1=mybir.AluOpType.add)
            nc.vector.tensor_mul(r_row, sc2, r0_row)
            return mu_row, r_row

        def broadcast_row(row):
            """GPSIMD-broadcast a [1, TPT] f32 row into a [128, TPT] sbuf tile."""
            bc = work.tile([128, TPT], f32, tag="bcrow")
            nc.gpsimd.partition_broadcast(bc, row)
            return bc

        ngroups = (ntiles + group - 1) // group
        for g in range(ngroups):
            tiles = range(g * group, min((g + 1) * group, ntiles))
            x2_tiles = {}
            # ------------- phase 1: LN1, qkv, attention, proj, LN2 -------------
            def stage_a(t):
                """DMA in + LN1 + xn (bf16). Returns per-tile dict."""
                c0 = t * TPT
                xt1 = xp.tile([128, TPT], f32, tag="xt1")
                xt2 = xp.tile([64, TPT], f32, tag="xt2")
                nc.sync.dma_start(out=xt1, in_=x_ext[0:128, c0:c0 + TPT])
                nc.sync.dma_start(out=xt2, in_=x_ext[128:DIM, c0:c0 + TPT])
                mu_row, r_row = ln_stats(xt1, xt2)
                mu_bc = broadcast_row(mu_row)
                xn1 = work.tile([128, TPT], f32, tag="xn1f")
                xn2_ = work.tile([64, TPT], f32, tag="xn2f")
                nc.gpsimd.tensor_sub(xn1, xt1, mu_bc[0:128, :])
                nc.gpsimd.tensor_sub(xn2_, xt2, mu_bc[0:64, :])
                r_bc = broadcast_row(r_row)
                xnb1 = work.tile([128, TPT], bf, tag="xnb1")
                xnb2 = work.tile([64, TPT], bf, tag="xnb2")
                nc.vector.tensor_mul(xnb1, xn1, r_bc[0:128, :])
                nc.vector.tensor_mul(xnb2, xn2_, r_bc[0:64, :])
                return dict(xt1=xt1, xt2=xt2, xnb1=xnb1, xnb2=xnb2)

            def stage_b(t, st_a):
                """qkv + attention + proj + residual -> (x2_1, x2_2)."""
                xt1, xt2 = st_a['xt1'], st_a['xt2']
                xnb1, xnb2 = st_a['xnb1'], st_a['xnb2']
                # ---- qkv: per-head base-0 layouts (all matmuls use PE row group 0;
                # different PSUM partition ranges come from the col position only,
                # since mixing row groups within one PSUM bank is illegal) ----
                def qkv_psum(wlo, whi):
                    pq = ps_mm.tile([128, 512], f32, tag="mm")
                    nc.tensor.matmul(pq[:, 0:TPT], wlo, xnb1, start=True, stop=False)
                    nc.tensor.matmul(pq[:, 0:TPT], whi, xnb2, start=False, stop=True)
                    return pq

                qall = qkp.tile([32, NH * TPT], bf, tag="qall")
                kall = qkp.tile([32, NH * TPT], bf, tag="kall")

                def evict_head(dst, dcol, psrc, prow, bias_ap, eng):
                    s = psrc[prow:prow + 32, 0:TPT]
                    d = dst[0:32, dcol:dcol + TPT]
                    if bias_ap is not None:
                        if eng == 'act':
                            nc.scalar.activation(d, s, A.Identity, bias=bias_ap)
                        else:
                            nc.vector.tensor_scalar_add(d, s, bias_ap)
                    else:
                        if eng == 'act':
                            nc.scalar.copy(d, s)
                        else:
                            nc.vector.tensor_copy(d, s)

                pa = qkv_psum(wa1, wa2)     # Q heads 0-3
                for h in range(4):
                    bap = biases['ba'][32 * h:32 * h + 32, 0:1] if 'ba' in biases else None
                    evict_head(qall, h * TPT, pa, 32 * h, bap, 'act' if h % 2 == 0 else 'vec')
                pb = qkv_psum(wb1, wb2)     # K heads 0-3
                for h in range(4):
                    bap = biases['bb'][32 * h:32 * h + 32, 0:1] if 'bb' in biases else None
                    evict_head(kall, h * TPT, pb, 32 * h, bap, 'act' if h % 2 == 1 else 'vec')
                pcd = qkv_psum(wcd1, wcd2)  # [Q4 Q5 K4 K5]
                for i, (dst, h) in enumerate(((qall, 4), (qall, 5), (kall, 4), (kall, 5))):
                    bap = biases['bcd'][32 * i:32 * i + 32, 0:1] if 'bcd' in biases else None
                    evict_head(dst, h * TPT, pcd, 32 * i, bap, 'act' if i % 2 == 0 else 'vec')

                # ---- V token-major: window pairs in one bank via col position ----
                vtiles = []
                for p in range(NPAIR):
                    vps = ps_sp.tile([128, 512], f32, tag="sp")
                    for wi, rb in ((2 * p, 0), (2 * p + 1, 64)):
                        cw = wi * N
                        nc.tensor.matmul(vps[rb:rb + N, 0:DIM], xnb1[:, cw:cw + N], wv1,
                                         start=True, stop=False, tile_position=(0, rb))
                        nc.tensor.matmul(vps[rb:rb + N, 0:DIM], xnb2[:, cw:cw + N], wv2,
                                         start=False, stop=True, tile_position=(0, rb))
                    vA = vp.tile([49, DIM], bf, tag="vt")
                    vB = vp.tile([49, DIM], bf, tag="vt")
                    nc.scalar.copy(vA, vps[0:N, 0:DIM])
                    nc.scalar.copy(vB, vps[64:64 + N, 0:DIM])
                    vtiles += [vA, vB]

                # ---- S^T + rpb bias, exp (per-window base-0 E tiles) ----
                etiles = []
                for p in range(NPAIR):
                    sps = ps_sp.tile([128, 512], f32, tag="sp")
                    for rb in (0, 64):
                        nc.tensor.matmul(sps[rb:rb + 64, 0:NH * N], i49_t, bsb_t,
                                         start=True, stop=False, tile_position=(0, rb),
                                         skip_group_check=True)
                    for wi, rb in ((2 * p, 0), (2 * p + 1, 64)):
                        cw = wi * N
                        for h in range(NH):
                            co = h * TPT + cw
                            nc.tensor.matmul(sps[rb:rb + N, N * h:N * h + N],
                                             kall[0:32, co:co + N], qall[0:32, co:co + N],
                                             start=False, stop=(h == NH - 1),
                                             tile_position=(0, rb),
                                             skip_group_check=True)
                    eA = epool.tile([64, NH * N], bf, tag="et")
                    eB = epool.tile([64, NH * N], bf, tag="et")
                    nc.scalar.activation(eA, sps[0:64, 0:NH * N], A.Exp)
                    nc.scalar.activation(eB, sps[64:128, 0:NH * N], A.Exp)
                    etiles += [eA, eB]

                # ---- AO + denominators (heads 0-3 then 4-5), all row group 0 ----
                def ao_block(h_lo, h_hi, parts):
                    aops_full = ps_ao.tile([128, 512], f32, tag="ao")
                    dps_full = ps_ao.tile([128, 512], f32, tag="ao")
                    aops = aops_full[0:parts, 0:TPT]
                    dps = dps_full[0:parts, 0:TPT]
                    for wi in range(WPT):
                        cg = wi * N
                        for h in range(h_lo, h_hi):
                            ho = 32 * (h - h_lo)
                            vs = vtiles[wi][0:N, 32 * h:32 * h + 32]
                            es = etiles[wi][0:N, N * h:N * h + N]
                            nc.tensor.matmul(aops[ho:ho + 32, cg:cg + N], vs, es,
                                             start=True, stop=True,
                                             tile_position=(0, ho))
                    for wi in range(WPT):
                        cg = wi * N
                        for h in range(h_lo, h_hi):
                            ho = 32 * (h - h_lo)
                            es = etiles[wi][0:N, N * h:N * h + N]
                            nc.tensor.matmul(dps[ho:ho + 32, cg:cg + N],
                                             ones_den[0:N, :], es,
                                             start=True, stop=True,
                                             tile_position=(0, ho))
                    rd = aop.tile([parts, TPT], f32, tag="rd")
                    nc.vector.reciprocal_approx_fast(rd, dps)
                    ao = aop.tile([parts, TPT], bf, tag="aosb")
                    nc.vector.tensor_mul(ao, aops, rd)
                    return ao

                ao1 = ao_block(0, 4, 128)
                ao2 = ao_block(4, 6, 64)


                # ---- proj + residual -> x2 (f32) ----
                x2_1 = x2p.tile([128, TPT], f32, tag="x2_1")
                x2_2 = x2p.tile([64, TPT], f32, tag="x2_2")
                for (m0, msz, xres, x2t) in ((0, 128, xt1, x2_1), (128, 64, xt2, x2_2)):
                    pp = ps_mm.tile([128, 512], f32, tag="mm")
                    nc.tensor.matmul(pp[0:msz, 0:TPT], wp1[:, m0:m0 + msz], ao1,
                                     start=True, stop=False)
                    nc.tensor.matmul(pp[0:msz, 0:TPT], wp2[:, m0:m0 + msz], ao2,
                                     start=False, stop=True)
                    if 'bp' in biases:
                        tmp = work.tile([msz, TPT], f32, tag="bptmp%d" % m0)
                        nc.scalar.activation(tmp, pp[0:msz, 0:TPT], A.Identity,
                                             bias=biases['bp'][0:msz, m0 // 128:m0 // 128 + 1])
                        nc.vector.tensor_add(x2t, tmp, xres)
                    else:
                        nc.vector.tensor_add(x2t, pp[0:msz, 0:TPT], xres)


                return x2_1, x2_2

            def stage_c(t, x2_1, x2_2):
                # ---- LN2 ----
                mu2_row, r2_row = ln_stats(x2_1, x2_2)
                mu2_bc = broadcast_row(mu2_row)
                s1 = work.tile([128, TPT], f32, tag="xn1f")
                s2 = work.tile([64, TPT], f32, tag="xn2f")
                nc.vector.tensor_sub(s1, x2_1, mu2_bc[0:128, 0:TPT])
                nc.vector.tensor_sub(s2, x2_2, mu2_bc[0:64, 0:TPT])
                r2_bc = broadcast_row(r2_row)
                xn2b1 = x2p.tile([128, TPT], bf, tag="xn2b1")
                xn2b2 = x2p.tile([64, TPT], bf, tag="xn2b2")
                nc.vector.tensor_mul(xn2b1, s1, r2_bc[0:128, 0:TPT])
                nc.vector.tensor_mul(xn2b2, s2, r2_bc[0:64, 0:TPT])

                x2_tiles[t] = (x2_1, x2_2, xn2b1, xn2b2)



            tl = list(tiles)
            a_st = {}
            for i, t in enumerate(tl):
                if i == 0:
                    a_st[t] = stage_a(t)
                    if len(tl) > 1:
                        a_st[tl[1]] = stage_a(tl[1])
                x2_1, x2_2 = stage_b(t, a_st.pop(t))
                if i + 2 < len(tl):
                    a_st[tl[i + 2]] = stage_a(tl[i + 2])
                stage_c(t, x2_1, x2_2)

            # ------------------- phase 2: MLP (gelu ACT table) -------------------
            for t in tiles:
                c0 = t * TPT
                x2_1, x2_2, xn2b1, xn2b2 = x2_tiles.pop(t)
                gt = gpool.tile([128, 6 * TPT], bf, tag="gt")
                for j in range(6):
                    pf = ps_mm.tile([128, 512], f32, tag="mm")
                    nc.tensor.matmul(pf[:, 0:TPT], w11[:, 128 * j:128 * (j + 1)], xn2b1,
                                     start=True, stop=False)
                    nc.tensor.matmul(pf[:, 0:TPT], w12[:, 128 * j:128 * (j + 1)], xn2b2,
                                     start=False, stop=True)
                    if 'b1' in biases:
                        nc.scalar.activation(gt[:, TPT * j:TPT * (j + 1)], pf[:, 0:TPT], GELU_F,
                                             bias=biases['b1'][:, j:j + 1])
                    else:
                        nc.scalar.activation(gt[:, TPT * j:TPT * (j + 1)], pf[:, 0:TPT], GELU_F)
                for (m0, msz, x2t) in ((0, 128, x2_1), (128, 64, x2_2)):
                    pf2 = ps_mm.tile([128, 512], f32, tag="mm")
                    for j in range(6):
                        nc.tensor.matmul(pf2[0:msz, 0:TPT],
                                         w2sb[:, DIM * j + m0:DIM * j + m0 + msz],
                                         gt[:, TPT * j:TPT * (j + 1)],
                                         start=(j == 0), stop=(j == 5))
                    ot = opool.tile([msz, TPT], f32, tag="ot%d" % m0)
                    if 'b2' in biases:
                        tmp2 = work.tile([msz, TPT], f32, tag="b2tmp%d" % m0)
                        nc.scalar.activation(tmp2, pf2[0:msz, 0:TPT], A.Identity,
                                             bias=biases['b2'][0:msz, m0 // 128:m0 // 128 + 1])
                        nc.vector.tensor_add(ot, tmp2, x2t)
                    else:
                        nc.vector.tensor_add(ot, pf2[0:msz, 0:TPT], x2t)
                    nc.sync.dma_start(out=out_ext[m0:m0 + msz, c0:c0 + TPT], in_=ot)
    nc.finalize()
    return nc


def _permute_in(x):
    """[B, C, H, W] f32 -> list of per-core [C, NWIN*N] window-ordered arrays."""
    xw = x.reshape(B, DIM, H // WS, WS, W // WS, WS).transpose(0, 1, 2, 4, 3, 5)
    xw = np.ascontiguousarray(xw).reshape(B, DIM, NWIN * N)
    return [np.ascontiguousarray(xw[b]) for b in range(B)]


def _permute_out(cores):
    """list of [C, NWIN*N] -> [B, C, H, W]."""
    o = np.stack(cores, 0).reshape(B, DIM, H // WS, W // WS, WS, WS)
    o = o.transpose(0, 1, 2, 4, 3, 5)
    return np.ascontiguousarray(o).reshape(B, DIM, H, W)


def run_kernel(inputs, trace=False, tmpdir=None):
    """Build + run. Returns (out [B,C,H,W], exec_time_ns or None)."""
    from concourse.bass_utils import run_bass_kernel_spmd

    x = np.asarray(inputs['x'], np.float32)
    wts, flags = prep_weights(inputs)
    nc = build_program(NTILES, GROUP, flags)
    xs = _permute_in(x)
    in_maps = [dict(wts, x=xs[b]) for b in range(B)]
    res = run_bass_kernel_spmd(nc, in_maps, core_ids=list(range(B)),
                               trace=trace, tmpdir=tmpdir)
    outs = [np.asarray(res.results[b]['out'], np.float32) for b in range(B)]
    return _permute_out(outs), res.exec_time_ns


def kernel(**inputs):
    out, _ = run_kernel(inputs, trace=False)
    return out


if __name__ == "__main__":
    import reference
    inputs = {k: np.asarray(v) for k, v in reference.setup_inputs().items()}
    got = kernel(**inputs)
    print("kernel output", got.shape, got.dtype)

